# revision 9
# baseline (speedup 1.0000x reference)
"""Bass/Trainium2 kernel for nn_Decoder (Bahdanau-attention LSTM decoder).

B=512, T-1=256, E=256, D=256. Data-parallel over 8 cores (64 batches each),
each core runs 2 independent batch-groups of 32 (software pipelining so one
group's serial tail hides under the other group's big ACT work).

Key algebraic restructuring vs the reference:
  - context only feeds the recurrence through the scalar fc_w . context, so
    xf[t,b] = sum_e fc_w[e] x[b,t,e] is precomputed (host) and
    y_tilde = (sum_t exp(s_tb) xf[t,b]) / (sum_t exp(s_tb)) + fc_w[E] y_t + fc_b
    (softmax division deferred; no max-subtraction needed since |scores|<~10).
  - full context materialized only once, after the last step.
  - sigmoid(x) = 0.5 tanh(x/2) + 0.5 so ACT only ever needs {tanh, exp}
    (one table set, no per-step table reloads). The 0.5 input scaling is
    folded into the i/f/o rows of W_hh / W_ih / biases on the host.
  - LSTM biases folded into an augmented K=2 outer-product matmul with
    rhs = [y_tilde; 1].

Engine mapping per step per group:
  PE : hcp = w1_hc^T[h;c] ; w2-reduce of tanh (quadrant-placed M=1 chunks);
       selector-matmul PSUM compaction; softmax sum via ones-matmul; gates.
  DVE: bf16 broadcast-add enc_proj + hcp (2x mode); psum bank copies;
       softmax smalls; LSTM elementwise.
  ACT: the big tanh; exp; gate tanh; tanh(c).
"""

import os
import numpy as np
import ml_dtypes

B, TM1, E, D = 512, 256, 256, 256
NCORES = 8
BC = B // NCORES          # 64 batches per core
G = 2                     # pipeline groups per core
BG = BC // G              # 32 batches per group
NCH = TM1 * BG // 2048    # 4 big-op chunks per (group, e-tile)
NSUB = 2048 // 512        # 4 w2 sub-chunks per big chunk
NCK = TM1 * BG // 512     # 16 score chunks per group
NBANK = 4                 # psum banks used for score chunks per group
STEPS_PER_ITER = 4
PEEL = 4

_RUNNER = None


# ----------------------------------------------------------------- host prep

def _host_prep(inputs):
    x = np.ascontiguousarray(np.asarray(inputs["input_encoded"], np.float32))
    y = np.ascontiguousarray(np.asarray(inputs["y_history"], np.float32))[..., 0]
    w1 = np.asarray(inputs["attn_w1"], np.float32)
    b1 = np.asarray(inputs["attn_b1"], np.float32)
    w2 = np.asarray(inputs["attn_w2"], np.float32)[0]
    W_ih = np.asarray(inputs["W_ih"], np.float32)[:, 0]
    W_hh = np.asarray(inputs["W_hh"], np.float32)
    bsum = (np.asarray(inputs["b_ih"], np.float32)
            + np.asarray(inputs["b_hh"], np.float32))
    fc_w = np.asarray(inputs["fc_w"], np.float32)[0]
    fc_b = float(np.asarray(inputs["fc_b"], np.float32)[0])
    fcf_w = np.asarray(inputs["fcf_w"], np.float32)
    fcf_b = np.asarray(inputs["fcf_b"], np.float32)

    def col_blocks(a):
        # (K, N) with K = k*128 partitions -> (128, k*N), col block j = rows 128j
        k = a.shape[0] // 128
        return np.ascontiguousarray(
            a.reshape(k, 128, a.shape[1]).transpose(1, 0, 2).reshape(128, -1))

    w1hcT = col_blocks(np.ascontiguousarray(w1[:, :2 * D].T))    # (128, 4*256)
    w1encT = col_blocks(np.ascontiguousarray(w1[:, 2 * D:].T))   # (128, 2*256)

    # gates: scale i,f,o pre-activations by 0.5 (sigmoid-via-tanh trick)
    scale = np.ones(4 * D, np.float32)
    scale[0 * D:2 * D] = 0.5     # i, f
    scale[3 * D:4 * D] = 0.5     # o
    whhT = col_blocks(np.ascontiguousarray((W_hh * scale[:, None]).T))  # (128, 2*1024)
    wihb = np.stack([W_ih * scale, bsum * scale]).astype(np.float32)    # (2, 1024)

    w2c = np.ascontiguousarray(w2.reshape(2, 128).T)             # (128, 2)
    fcfT = col_blocks(np.ascontiguousarray(fcf_w.T))             # (128, 4*2)
    fcfb = fcf_b.reshape(2, 1)
    b1rep = np.ascontiguousarray(
        np.concatenate([np.tile(b1[:128, None], (1, BG)),
                        np.tile(b1[128:, None], (1, BG))], axis=1))  # (128, 64)

    selc = np.zeros((128, NBANK, NCK), np.float32)
    for c in range(NCK):
        k, q = c // 4, c % 4
        selc[32 * q, k, c] = 1.0
    selc = np.ascontiguousarray(selc.reshape(128, NBANK * NCK))  # (128, 64)
    ones16 = np.ones((16, 1), np.float32)
    ident = np.eye(128, dtype=np.float32)

    # per-core tensors
    xs, xfs, yts = [], [], []
    for i in range(NCORES):
        xc = x[i * BC:(i + 1) * BC]                      # (64, 256, 256)
        yc = y[i * BC:(i + 1) * BC]                      # (64, 256)
        xf = xc @ fc_w[:E]                               # (64, 256)
        # xf_g layout (2 groups, 16, 512): [g, c, tl*32+b] = xf[32g+b, 16c+tl]
        xfg = np.zeros((G, 16, 512), np.float32)
        yt = np.zeros((G, TM1 * BG), np.float32)
        for g in range(G):
            xg = xf[g * BG:(g + 1) * BG]                 # (32, 256) [b, t]
            xfg[g] = xg.T.reshape(16, 16, BG).transpose(0, 1, 2).reshape(16, 512)
            yg = yc[g * BG:(g + 1) * BG]                 # (32, 256)
            yt[g] = (fc_w[E] * yg.T + fc_b).reshape(-1)  # [t*32+b]
        xs.append(np.ascontiguousarray(xc.reshape(BC * TM1, E)))
        xfs.append(np.ascontiguousarray(xfg.reshape(G * 16, 512)))
        yts.append(np.ascontiguousarray(yt.astype(ml_dtypes.bfloat16)))

    consts = dict(w1hcT=w1hcT, w1encT=w1encT, whhT=whhT, wihb=wihb, w2c=w2c,
                  fcfT=fcfT, fcfb=fcfb, b1rep=b1rep, selc=selc, ones16=ones16,
                  ident=ident)
    return consts, xs, xfs, yts


# ------------------------------------------------------------ kernel builder

def build_nc(num_steps=TM1, nc_kwargs=None):
    import concourse.bass as bass
    import concourse.mybir as mybir
    from concourse import tile

    f32 = mybir.dt.float32
    f32r = mybir.dt.float32r
    bf16 = mybir.dt.bfloat16
    ADD = mybir.AluOpType.add
    MUL = mybir.AluOpType.mult
    TANH = mybir.ActivationFunctionType.Tanh
    EXP = mybir.ActivationFunctionType.Exp
    AX = mybir.AxisListType.X

    nc = bass.Bass("TRN2", **(nc_kwargs or {}))

    x_d = nc.dram_tensor("x", [BC * TM1, E], f32, kind="ExternalInput")
    xf_d = nc.dram_tensor("xf", [G * 16, 512], f32, kind="ExternalInput")
    yt_d = nc.dram_tensor("ytail", [G, TM1 * BG], bf16, kind="ExternalInput")
    w1hcT_d = nc.dram_tensor("w1hcT", [128, 4 * 256], f32, kind="ExternalInput")
    w1encT_d = nc.dram_tensor("w1encT", [128, 2 * 256], f32, kind="ExternalInput")
    whhT_d = nc.dram_tensor("whhT", [128, 2 * 1024], f32, kind="ExternalInput")
    wihb_d = nc.dram_tensor("wihb", [2, 1024], f32, kind="ExternalInput")
    w2c_d = nc.dram_tensor("w2c", [128, 2], f32, kind="ExternalInput")
    fcfT_d = nc.dram_tensor("fcfT", [128, 8], f32, kind="ExternalInput")
    fcfb_d = nc.dram_tensor("fcfb", [2, 1], f32, kind="ExternalInput")
    b1rep_d = nc.dram_tensor("b1rep", [128, 2 * BG], f32, kind="ExternalInput")
    selc_d = nc.dram_tensor("selc", [128, NBANK * NCK], f32, kind="ExternalInput")
    ones_d = nc.dram_tensor("ones16", [16, 1], f32, kind="ExternalInput")
    ident_d = nc.dram_tensor("ident", [128, 128], f32, kind="ExternalInput")
    out_d = nc.dram_tensor("out", [2, BC], f32, kind="ExternalOutput")

    # gate M-tile order [i0 i1 f0 f1 o0 o1 g0 g1] -> 4D row starts
    RGATE = [0, 128, 256, 384, 768, 896, 512, 640]

    with tile.TileContext(nc) as tc:
      with tc.tile_pool(name="const", bufs=1) as cp, \
           tc.tile_pool(name="state", bufs=1) as st, \
           tc.tile_pool(name="psum", bufs=1, space="PSUM") as pp:

        # ---- persistent psum banks
        P8 = [pp.tile([128, 512], f32, tag=f"bank{i}", name=f"bank{i}") for i in range(8)]
        sc_banks = P8[0:4]
        cmp_ps = P8[4][0:16, :]
        nd_ps = P8[4][64:65, 0:64]       # [den(32) | num(32)]
        gates_ps = [P8[5][:, 0:256], P8[6][:, 0:256]]
        hcp_ps = [P8[5][:, 256:320], P8[6][:, 256:320]]
        ep_mm_ps = P8[7]

        # ---- consts to SBUF
        def load(dram, shape, nm, dtype=f32):
            t = cp.tile(shape, dtype, name=nm, tag=nm)
            nc.sync.dma_start(t[:], dram[:])
            return t

        w1hcT = load(w1hcT_d, [128, 4 * 256], "c_w1hcT")
        w1encT = load(w1encT_d, [128, 2 * 256], "c_w1encT")
        whhT = load(whhT_d, [128, 2 * 1024], "c_whhT")
        wihb = load(wihb_d, [2, 1024], "c_wihb")
        w2c = load(w2c_d, [128, 2], "c_w2c")
        fcfT = load(fcfT_d, [128, 8], "c_fcfT")
        fcfb = load(fcfb_d, [2, 1], "c_fcfb")
        b1rep = load(b1rep_d, [128, 2 * BG], "c_b1rep")
        selc = load(selc_d, [128, NBANK * NCK], "c_selc")
        ones16 = load(ones_d, [16, 1], "c_ones16")
        ident = load(ident_d, [128, 128], "c_ident")
        xf = []
        for g in range(G):
            t = cp.tile([16, 512], f32, name=f"c_xf{g}", tag=f"c_xf{g}")
            nc.sync.dma_start(t[:], xf_d[16 * g:16 * (g + 1), :])
            xf.append(t)
        ytail = []
        for g in range(G):
            t = cp.tile([1, TM1 * BG], bf16, name=f"c_yt{g}", tag=f"c_yt{g}")
            nc.sync.dma_start(t[:], yt_d[g:g + 1, :])
            ytail.append(t)

        # ---- persistent state
        h_g = [st.tile([128, 2 * BG], f32, tag=f"h{g}", name=f"h{g}") for g in range(G)]
        c_g = [st.tile([128, 2 * BG], f32, tag=f"c{g}", name=f"c{g}") for g in range(G)]
        y_aug = [st.tile([2, BG], f32, tag=f"ya{g}", name=f"ya{g}") for g in range(G)]
        EPb = [[st.tile([128, TM1 * BG], bf16, tag=f"ep{g}{et}", name=f"ep{g}{et}")
                for et in range(2)] for g in range(G)]
        exp_fin = [st.tile([16, 512], f32, tag=f"ef{g}", name=f"ef{g}") for g in range(G)]
        r_fin = st.tile([1, BC], f32, tag="rfin")

        for g in range(G):
            nc.vector.memset(h_g[g][:], 0.0)
            nc.vector.memset(c_g[g][:], 0.0)
            nc.vector.memset(y_aug[g][:], 1.0)
        for t in P8:
            nc.vector.memset(t[:], 0.0)

        # bf16 copy of w1enc^T (EP matmuls run bf16 for 1 cyc/row)
        w1encTb = cp.tile([128, 2 * 256], bf16)
        nc.vector.tensor_copy(w1encTb[:], w1encT[:])

        # =========================== precompute: EP = x @ w1enc^T (on device)
        with tc.tile_pool(name="stage", bufs=1) as sgp, \
             tc.tile_pool(name="xin", bufs=6) as xin:
            for g in range(G):
                xT = [sgp.tile([128, BG * TM1], bf16, tag=f"xT{et}", name=f"xT{et}")
                      for et in range(2)]
                # transpose x rows for this group: 64 natural tiles of (128,256)
                for i in range(BG * TM1 // 128):
                    xa = xin.tile([128, E], f32, tag="xa")
                    nc.sync.dma_start(
                        xa[:], x_d[g * BG * TM1 + i * 128:
                                   g * BG * TM1 + (i + 1) * 128, :])
                    b_i, th = i // 2, i % 2
                    for et in range(2):
                        trp = sc_banks[(2 * i + et) % 4][0:128, 0:128]
                        nc.tensor.transpose(trp, xa[:, 128 * et:128 * (et + 1)],
                                            ident[:])
                        dst = xT[et][:, b_i * TM1 + th * 128:
                                     b_i * TM1 + th * 128 + 128]
                        if (2 * i + et) % 2 == 0:
                            nc.vector.tensor_copy(dst, trp)
                        else:
                            nc.scalar.copy(dst, trp)
                # EP matmuls: chunk c = t in [16c,16c+16) x all 32 b
                for et in range(2):
                    for c in range(NCK):
                        ep_out = P8[4 + (c % 4)][:, 0:512]
                        for eint in range(2):
                            lhsT = w1encTb[:, 256 * eint + 128 * et:
                                           256 * eint + 128 * et + 128]
                            rhs = (xT[eint][:]
                                   .rearrange("p (b t) -> p b t", b=BG)
                                   [:, :, 16 * c:16 * c + 16]
                                   .rearrange("p b t -> p t b"))
                            nc.tensor.matmul(
                                ep_out, lhsT, rhs,
                                start=(eint == 0), stop=(eint == 1))
                        dst = EPb[g][et][:, 512 * c:512 * (c + 1)]
                        if c % 2 == 0:
                            nc.vector.tensor_copy(dst, ep_out)
                        else:
                            nc.scalar.copy(dst, ep_out)

        # zero the score banks again (transposes dirtied them)
        for t in sc_banks:
            nc.vector.memset(t[:], 0.0)

        # ============================================ one decoder step
        with tc.tile_pool(name="work", bufs=4) as wk, \
             tc.tile_pool(name="tanhp", bufs=3) as tp, \
             tc.tile_pool(name="small", bufs=2) as sm:

            def step(g, ytail_slice, last=False):
                # 1. hcp = w1_hc^T [h;c] + b1
                for et in range(2):
                    for j in range(4):
                        lhsT = w1hcT[:, 256 * j + 128 * et:
                                     256 * j + 128 * et + 128]
                        src = h_g[g] if j < 2 else c_g[g]
                        rhs = src[:, BG * (j % 2):BG * (j % 2) + BG]
                        nc.tensor.matmul(hcp_ps[g][:, BG * et:BG * et + BG],
                                         lhsT, rhs,
                                         start=(j == 0), stop=(j == 3))
                hcp_f = sm.tile([128, 2 * BG], f32, tag="hcpf")
                nc.vector.tensor_tensor(hcp_f[:], hcp_ps[g], b1rep[:], ADD)
                hcp_b = sm.tile([128, 2 * BG], bf16, tag="hcpb")
                nc.vector.tensor_copy(hcp_b[:], hcp_f[:])

                # 2. big add + tanh + w2-reduce, chunked
                copied = [False] * NBANK
                tanh_ch = {}
                for ch in range(NCH):
                    for et in range(2):
                        pre = wk.tile([128, 2048], bf16, tag="pre")
                        ep3 = (EPb[g][et][:, 2048 * ch:2048 * (ch + 1)]
                               .rearrange("p (t b) -> p t b", b=BG))
                        hb = (hcp_b[:, BG * et:BG * et + BG]
                              .unsqueeze(1).broadcast_to([128, 64, BG]))
                        nc.vector.tensor_tensor(
                            pre[:].rearrange("p (t b) -> p t b", b=BG),
                            ep3, hb, ADD)
                        th = tp.tile([128, 2048], f32, tag="th")
                        nc.scalar.activation(th[:], pre[:], TANH)
                        tanh_ch[et] = th
                    for j in range(NSUB):
                        c = NSUB * ch + j
                        k, q = c // 4, c % 4
                        for et in range(2):
                            nc.tensor.matmul(
                                sc_banks[k][32 * q:32 * q + 1, :],
                                w2c[:, et:et + 1].bitcast(f32r),
                                tanh_ch[et][:, 512 * j:512 * (j + 1)]
                                .bitcast(f32r),
                                start=(et == 0), stop=(et == 1),
                                tile_position=(0, 32 * q))
                    # bank k complete once its 4 quadrants written
                    k_done = ch  # chunks 4k..4k+3 fill bank k = ch
                    if not copied[k_done]:
                        copied[k_done] = True
                        scsp = wk.tile([128, 512], f32, tag="scsp")
                        nc.vector.tensor_copy(scsp[:], sc_banks[k_done][:])
                        nc.tensor.matmul(
                            cmp_ps, selc[:, 16 * k_done:16 * (k_done + 1)]
                            .bitcast(f32r),
                            scsp[:].bitcast(f32r),
                            start=(k_done == 0), stop=(k_done == NBANK - 1))

                # 3. softmax (no max-sub), deferred division
                expg = exp_fin[g] if last else sm.tile([16, 512], f32, tag="expg", name="expg")
                nc.scalar.activation(expg[:], cmp_ps, EXP)
                prod = sm.tile([16, 512], f32, tag="prod")
                nc.vector.tensor_tensor(prod[:], expg[:], xf[g][:], MUL)
                ndin = sm.tile([16, 2 * BG], f32, tag="ndin")
                nc.vector.tensor_reduce(
                    ndin[:, 0:BG],
                    expg[:].rearrange("p (t b) -> p b t", b=BG), AX, ADD)
                nc.vector.tensor_reduce(
                    ndin[:, BG:2 * BG],
                    prod[:].rearrange("p (t b) -> p b t", b=BG), AX, ADD)
                nc.tensor.matmul(nd_ps, ones16[:], ndin[:],
                                 start=True, stop=True, tile_position=(0, 64))
                rcp = r_fin[0:1, BG * g:BG * g + BG] if last \
                    else sm.tile([1, BG], f32, tag="rcp", name="rcp")[:]
                nc.vector.reciprocal(rcp, nd_ps[0:1, 0:BG])
                ytl = sm.tile([1, BG], f32, tag="ytl")
                nc.vector.tensor_tensor(ytl[:], nd_ps[0:1, BG:2 * BG], rcp, MUL)
                nc.vector.tensor_tensor(y_aug[g][0:1, :], ytl[:],
                                        ytail_slice, ADD)

                # 4. gates
                for m in range(8):
                    r0 = RGATE[m]
                    for dt in range(2):
                        nc.tensor.matmul(
                            gates_ps[g][:, 32 * m:32 * m + 32],
                            whhT[:, 1024 * dt + r0:1024 * dt + r0 + 128],
                            h_g[g][:, BG * dt:BG * dt + BG],
                            start=(dt == 0), stop=False)
                    nc.tensor.matmul(
                        gates_ps[g][:, 32 * m:32 * m + 32],
                        wihb[:, r0:r0 + 128], y_aug[g][:],
                        start=False, stop=True)
                tg = sm.tile([128, 256], f32, tag="tg")
                nc.scalar.activation(tg[:], gates_ps[g], TANH)
                sg_ = sm.tile([128, 192], f32, tag="sg")
                nc.vector.tensor_scalar(sg_[:], tg[:, 0:192], 0.5, 0.5, MUL, ADD)

                # 5. LSTM update (i=sg[0:64], f=sg[64:128], o=sg[128:192], g=tg[192:256])
                t1 = sm.tile([128, 2 * BG], f32, tag="t1")
                nc.vector.tensor_tensor(t1[:], sg_[:, 64:128], c_g[g][:], MUL)
                t2 = sm.tile([128, 2 * BG], f32, tag="t2")
                nc.vector.tensor_tensor(t2[:], sg_[:, 0:64], tg[:, 192:256], MUL)
                nc.vector.tensor_tensor(c_g[g][:], t1[:], t2[:], ADD)
                tc_ = sm.tile([128, 2 * BG], f32, tag="tc")
                nc.scalar.activation(tc_[:], c_g[g][:], TANH)
                nc.vector.tensor_tensor(h_g[g][:], sg_[:, 128:192], tc_[:], MUL)

            # ---- main loop + peel
            n_iter = (num_steps - PEEL) // STEPS_PER_ITER
            if n_iter > 0:
                with tc.For_i(0, n_iter, 1) as iv:
                    for k in range(STEPS_PER_ITER):
                        for g in range(G):
                            sl = ytail[g][0:1,
                                          bass.ts(iv * STEPS_PER_ITER + k, BG)]
                            step(g, sl)
            for s in range(num_steps - PEEL, num_steps):
                for g in range(G):
                    sl = ytail[g][0:1, s * BG:(s + 1) * BG]
                    step(g, sl, last=(s == num_steps - 1))

            # =================================== final: context + fcf output
            # alpha-weighted context, division deferred via r_fin
            rep = st.tile([128, BC], f32, tag="rep")
            nc.gpsimd.partition_broadcast(rep[:], r_fin[:])
            # exp re-layout (16,512) -> (t(128) x b(32)) tiles via small DMAs
            expt = [[st.tile([128, BG], f32, tag=f"et{g}{tt}", name=f"et{g}{tt}")
                     for tt in range(2)] for g in range(G)]
            for g in range(G):
                for tt in range(2):
                    for i in range(8):
                        src = (exp_fin[g][8 * tt + i:8 * tt + i + 1, :]
                               .rearrange("p (t b) -> p t b", b=BG))
                        nc.sync.dma_start(
                            expt[g][tt][16 * i:16 * (i + 1), :], src)
            ctx_ps = [P8[5][:, 0:BC], P8[6][:, 0:BC]]
            with tc.tile_pool(name="xb", bufs=4) as xb:
                for b in range(BC):
                    g, bl = b // BG, b % BG
                    for tt in range(2):
                        xbt = xb.tile([128, E], f32, tag="xbt")
                        nc.sync.dma_start(
                            xbt[:], x_d[b * TM1 + 128 * tt:
                                        b * TM1 + 128 * (tt + 1), :])
                        for et in range(2):
                            nc.tensor.matmul(
                                ctx_ps[et][:, b:b + 1],
                                xbt[:, 128 * et:128 * (et + 1)].bitcast(f32r),
                                expt[g][tt][:, bl:bl + 1].bitcast(f32r),
                                start=(tt == 0), stop=(tt == 1))
            ctx_sb = [st.tile([128, BC], f32, tag=f"ctx{et}", name=f"ctx{et}") for et in range(2)]
            for et in range(2):
                nc.vector.tensor_tensor(ctx_sb[et][:], ctx_ps[et], rep[:], MUL)
            # assemble h (d-tiles x all b)
            h_all = [st.tile([128, BC], f32, tag=f"ha{dt}", name=f"ha{dt}") for dt in range(2)]
            for dt in range(2):
                for g in range(G):
                    nc.vector.tensor_copy(
                        h_all[dt][:, BG * g:BG * (g + 1)],
                        h_g[g][:, BG * dt:BG * dt + BG])
            out_ps = P8[7][0:2, 0:BC]
            state = [h_all[0], h_all[1], ctx_sb[0], ctx_sb[1]]
            for kk in range(4):
                nc.tensor.matmul(out_ps, fcfT[:, 2 * kk:2 * kk + 2],
                                 state[kk][:], start=(kk == 0), stop=(kk == 3))
            out_sb = st.tile([2, BC], f32, tag="outsb")
            nc.vector.tensor_scalar(out_sb[:], out_ps, fcfb[:], None, ADD)
            nc.sync.dma_start(out_d[:], out_sb[:])

    return nc


# ---------------------------------------------------------------- execution

def kernel(**inputs):
    global _RUNNER
    from concourse.bass_utils import run_bass_kernel_spmd
    consts, xs, xfs, yts = _host_prep(inputs)
    if _RUNNER is None:
        _RUNNER = build_nc(TM1)
    in_maps = []
    for i in range(NCORES):
        m = dict(consts)
        m["x"] = xs[i]
        m["xf"] = xfs[i]
        m["ytail"] = yts[i]
        in_maps.append(m)
    res = run_bass_kernel_spmd(_RUNNER, in_maps, core_ids=list(range(NCORES)))
    out = np.zeros((B, 2), np.float32)
    for i in range(NCORES):
        o = res.results[i]["out"]          # (2, 64)
        out[i * BC:(i + 1) * BC, :] = np.asarray(o, np.float32).T
    return out
